# revision 11
# baseline (speedup 1.0000x reference)
"""GAT (3-layer, 4-head) on 8 Trainium2 NeuronCores.

Sharding: nodes padded to 100352 = 8 * 98 * 128; core c owns the contiguous
dst-node range [c*12544, (c+1)*12544) and its incoming-edge CSR slice.

Per layer: dense phase computes [feat | el | er] rows in one PSUM matmul
(hT stationary, Wext = [W | W@a_l | W@a_r] moving, bf16), writes packed
bf16 rows to DRAM; one AllGather per layer shares the table; the
aggregation phase gathers each dst tile's 16 neighbor rows with 16
single-offset indirect DMAs (the HW SWDGE indirect path supports one
offset per partition per instruction — multi-offset APs only work in the
simulator), then does edge-softmax + weighted sum on DVE. Layer l's
aggregation is fused per-tile with layer l+1's dense phase; gathers are
software-prefetched 3 tiles ahead of consumption.
"""
import os
import sys

sys.path.insert(0, "/opt/trn_rl_repo")

import numpy as np
import ml_dtypes

BF16 = ml_dtypes.bfloat16

P = 128
NCORES = 8
N = 100000
DEG = 16
HEADS = 4
HID = 64
IN_DIM = 256
NCLS = 41
NEG = 0.2

TPC = 98                  # dst tiles per core
NLOC = TPC * P            # 12544
NPAD = NCORES * NLOC      # 100352
D1 = HEADS * HID          # 256
D2 = HEADS * NCLS         # 164
ROW1 = D1 + 16            # packed row: 256 feat + 4 el + pad (544B)
ROW2 = D2 + 12            # 164 feat + 4 el + pad (352B)

NCH = 2                   # AllGather chunks per layer
CHT = TPC // NCH          # 49 tiles per chunk
CHROWS = CHT * P          # 6272 rows per core per chunk
CHTOT = NCORES * CHROWS   # 50176 rows per chunk in the gathered table

PREF = 4                  # gather software-prefetch depth (tiles)

DL = [D1, D1, D2]
ROWL = [ROW1, ROW1, ROW2]


def _pack_a(al, ar, fdim, hdim):
    a = np.zeros((fdim, 8), np.float32)
    al = np.asarray(al, np.float32)
    ar = np.asarray(ar, np.float32)
    for h in range(HEADS):
        a[h * hdim:(h + 1) * hdim, h] = al[h]
        a[h * hdim:(h + 1) * hdim, 4 + h] = ar[h]
    return a


def build_program():
    import concourse.bass as bass
    import concourse.bacc as bacc
    import concourse.mybir as mybir
    import concourse.tile as tile
    from concourse.masks import make_identity

    f32 = mybir.dt.float32
    bf16 = mybir.dt.bfloat16
    i32 = mybir.dt.int32
    ADD = mybir.AluOpType.add
    MULT = mybir.AluOpType.mult
    MAX = mybir.AluOpType.max
    nc = bacc.Bacc("TRN2", target_bir_lowering=False, debug=False,
                   num_devices=NCORES)

    hT0 = nc.declare_dram_parameter("hT0", [P, TPC * 2 * P], bf16,
                                    isOutput=False)
    idx_in = nc.declare_dram_parameter("idx", [P, TPC * DEG], i32,
                                       isOutput=False)
    Wx = [nc.declare_dram_parameter(f"Wx{l}", [2 * P, DL[l] + 8], bf16,
                                    isOutput=False) for l in range(3)]
    out_ext = nc.declare_dram_parameter("out", [NLOC, NCLS], f32,
                                        isOutput=True)

    rg = [list(range(NCORES))]

    with tile.TileContext(nc) as tc:
        with (
            tc.tile_pool(name="const", bufs=1) as cp,
            tc.tile_pool(name="resid", bufs=1) as rp,
            tc.tile_pool(name="wk", bufs=3) as wk,
            tc.tile_pool(name="gat", bufs=6) as gp,
            tc.tile_pool(name="seq", bufs=2) as sq,
            tc.tile_pool(name="psp", bufs=2, space="PSUM") as psp,
            tc.tile_pool(name="dram", bufs=1, space="DRAM") as dram,
        ):
            identb = cp.tile([P, P], bf16)
            make_identity(nc, identb[:])

            # weights resident in SBUF: wsb[l][ic] : [128, DL+8] bf16
            wsb = []
            for l in range(3):
                wl = []
                for ic in range(2):
                    w = cp.tile([P, DL[l] + 8], bf16, name=f"w{l}_{ic}")
                    nc.sync.dma_start(out=w[:],
                                      in_=Wx[l][ic * P:(ic + 1) * P, :])
                    wl.append(w)
                wsb.append(wl)

            # resident buffers
            h_res = rp.tile([P, TPC * D1], bf16)            # 6.3 MB
            er_res = [rp.tile([P, TPC * 4], bf16, name=f"er{l}")
                      for l in range(3)]
            idxs = rp.tile([P, TPC * DEG], i32)
            nc.sync.dma_start(out=idxs[:], in_=idx_in[:])

            # DRAM: per-half AllGather inputs + per-layer gathered tables
            # (non-Shared: the Shared space enforces a single writer, and we
            # want two AGs per layer so the first overlaps fused compute)
            ag_in = [[dram.tile([CHROWS, ROWL[l]], bf16, name=f"ag{l}_{c}")
                      for c in range(NCH)] for l in range(3)]
            table = [dram.tile([NCH * CHTOT, ROWL[l]], bf16,
                               name=f"table{l}")
                     for l in range(3)]

            def dense_tile(l, t):
                """feat/el/er for dst tile t of layer l; writes packed row
                table chunk + er_res; fires the chunk AllGather."""
                DO = DL[l]
                ROW = ROWL[l]
                ht = wk.tile([P, 2 * P], bf16, tag="ht", name=f"ht{l}_{t}")
                if l == 0:
                    nc.sync.dma_start(out=ht[:],
                                      in_=hT0[:, t * 2 * P:(t + 1) * 2 * P])
                else:
                    for ic in range(2):
                        tp = psp.tile([P, P], bf16, tag="tp", bufs=2,
                                      name=f"tp{l}_{t}_{ic}")
                        nc.tensor.transpose(
                            tp[:],
                            h_res[:, t * D1 + ic * P: t * D1 + (ic + 1) * P],
                            identb[:])
                        nc.scalar.copy(ht[:, ic * P:(ic + 1) * P], tp[:])
                fp = psp.tile([P, 272], f32, tag="fp", bufs=2,
                              name=f"fp{l}_{t}")
                for ic in range(2):
                    nc.tensor.matmul(fp[:, :DO + 8],
                                     ht[:, ic * P:(ic + 1) * P],
                                     wsb[l][ic][:, :DO + 8],
                                     start=(ic == 0), stop=(ic == 1))
                packed = wk.tile([P, ROW1], bf16, tag="pk", name=f"pk{l}_{t}")
                nc.scalar.copy(packed[:, :DO + 4], fp[:, :DO + 4])
                nc.scalar.copy(er_res[l][:, t * 4:(t + 1) * 4],
                               fp[:, DO + 4:DO + 8])
                c = t // CHT
                r = (t % CHT) * P
                nc.sync.dma_start(out=ag_in[l][c][r:r + P, :],
                                  in_=packed[:, :ROW])
                if t % CHT == CHT - 1:
                    nc.gpsimd.collective_compute(
                        "AllGather", mybir.AluOpType.bypass,
                        replica_groups=rg,
                        ins=[ag_in[l][c][:]],
                        outs=[table[l][c * CHTOT:(c + 1) * CHTOT, :]])

            def emit_gather(l, t):
                ROW = ROWL[l]
                G = gp.tile([P, DEG * ROW1], bf16, tag="G", name=f"G{l}_{t}")
                for k in range(DEG):
                    nc.gpsimd.indirect_dma_start(
                        out=G[:, k * ROW:(k + 1) * ROW],
                        out_offset=None,
                        in_=table[l][:],
                        in_offset=bass.IndirectOffsetOnAxis(
                            ap=idxs[:, t * DEG + k:t * DEG + k + 1], axis=0),
                    )
                return G

            def agg_tile(l, t, G):
                """edge softmax + weighted aggregation for dst tile t."""
                DO = DL[l]
                ROW = ROWL[l]
                hd = DO // HEADS
                Gv = G[:, :DEG * ROW].rearrange("p (k r) -> p k r", k=DEG)
                # e[p, k, h] = el_src + er_dst  (k-major so last dim packed)
                e = wk.tile([P, 64], f32, tag="e", name=f"e{l}_{t}")
                ev = e[:].rearrange("p (k h) -> p k h", k=DEG)
                nc.vector.tensor_tensor(
                    out=ev, in0=Gv[:, :, DO:DO + 4],
                    in1=er_res[l][:, t * 4:(t + 1) * 4].unsqueeze(1)
                        .to_broadcast([P, DEG, 4]),
                    op=ADD)
                # leaky relu: e = max(NEG*e, e)
                nc.vector.scalar_tensor_tensor(
                    out=e[:], in0=e[:], scalar=NEG, in1=e[:],
                    op0=MULT, op1=MAX)
                # exp + per-head denominators (ACT accumulates the sum)
                ex = wk.tile([P, 64], bf16, tag="ex", name=f"ex{l}_{t}")
                den = wk.tile([P, 4], f32, tag="den", name=f"den{l}_{t}")
                ev2 = e[:].rearrange("p (k h) -> p k h", k=DEG)
                exv = ex[:].rearrange("p (k h) -> p k h", k=DEG)
                for h in range(HEADS):
                    nc.scalar.activation(
                        exv[:, :, h], ev2[:, :, h],
                        mybir.ActivationFunctionType.Exp,
                        accum_out=den[:, h:h + 1])
                rden = wk.tile([P, 4], f32, tag="rden", name=f"rd{l}_{t}")
                nc.vector.reciprocal(rden[:], den[:])
                if l == 2:
                    nc.vector.tensor_scalar_mul(rden[:], rden[:],
                                                1.0 / HEADS)
                alp = wk.tile([P, 64], bf16, tag="alp", name=f"al{l}_{t}")
                nc.vector.tensor_tensor(
                    out=alp[:].rearrange("p (k h) -> p k h", k=DEG),
                    in0=exv,
                    in1=rden[:].unsqueeze(1).to_broadcast([P, DEG, 4]),
                    op=MULT)
                # msg[p, k, h, d] = feat * alpha
                W0 = DEG * DO
                msg = sq.tile([P, DEG * D1], bf16, tag="msg",
                              name=f"ms{l}_{t}")
                nc.vector.tensor_tensor(
                    out=msg[:, :W0].rearrange("p (k h d) -> p k h d",
                                              k=DEG, h=HEADS),
                    in0=Gv[:, :, 0:DO].rearrange("p k (h d) -> p k h d",
                                                 h=HEADS),
                    in1=alp[:].rearrange("p (k h) -> p k h", k=DEG)
                        .to_broadcast([P, DEG, 4, hd]),
                    op=MULT)
                # tree reduce over k: DVE does the two wide levels (as
                # scalar_tensor_tensor for the 4x perf mode), GpSimd the tail
                w1 = W0 // 2
                s0 = sq.tile([P, 2048], bf16, tag="s0", name=f"s0{l}_{t}")
                nc.vector.scalar_tensor_tensor(
                    out=s0[:, :w1], in0=msg[:, 0:w1], scalar=1.0,
                    in1=msg[:, w1:W0], op0=MULT, op1=ADD)
                w2 = w1 // 2
                s1 = sq.tile([P, 1024], bf16, tag="s1", name=f"s1{l}_{t}")
                nc.vector.scalar_tensor_tensor(
                    out=s1[:, :w2], in0=s0[:, 0:w2], scalar=1.0,
                    in1=s0[:, w2:w1], op0=MULT, op1=ADD)
                w3 = w2 // 2
                s2 = sq.tile([P, 512], bf16, tag="s2", name=f"s2{l}_{t}")
                nc.vector.scalar_tensor_tensor(
                    out=s2[:, :w3], in0=s1[:, 0:w3], scalar=1.0,
                    in1=s1[:, w3:w2], op0=MULT, op1=ADD)
                w4 = w3 // 2
                if l < 2:
                    nc.vector.scalar_tensor_tensor(
                        out=h_res[:, t * D1:(t + 1) * D1],
                        in0=s2[:, 0:w4], scalar=1.0,
                        in1=s2[:, w4:w3], op0=MULT, op1=ADD)
                else:
                    cur = wk.tile([P, D2], bf16, tag="cur", name=f"cu{t}")
                    nc.vector.scalar_tensor_tensor(
                        out=cur[:], in0=s2[:, 0:w4], scalar=1.0,
                        in1=s2[:, w4:w3], op0=MULT, op1=ADD)
                    lg = wk.tile([P, NCLS], f32, tag="lg", name=f"lg{t}")
                    nc.vector.tensor_reduce(
                        out=lg[:],
                        in_=cur[:].rearrange("p (h c) -> p c h", h=HEADS),
                        axis=mybir.AxisListType.X, op=ADD)
                    nc.sync.dma_start(out=out_ext[t * P:(t + 1) * P, :],
                                      in_=lg[:])

            # ---- stage 0: dense layer 0 + chunked AllGather of table0 ----
            for t in range(TPC):
                dense_tile(0, t)

            # ---- stages 1..3: agg(l) fused with dense(l+1) ----
            for l in range(3):
                pend = {}
                for tt in range(PREF):
                    pend[tt] = emit_gather(l, tt)
                for t in range(TPC):
                    if t + PREF < TPC:
                        pend[t + PREF] = emit_gather(l, t + PREF)
                    G = pend.pop(t)
                    agg_tile(l, t, G)
                    if l < 2:
                        dense_tile(l + 1, t)

    nc.compile()
    return nc


def prep_inputs(row_ptr, col_ind, inputs, W0, al0, ar0, W1, al1, ar1,
                W2, al2, ar2):
    col = np.asarray(col_ind, np.int32).reshape(N, DEG)
    col_pad = np.zeros((NPAD, DEG), np.int32)
    col_pad[:N] = col
    # half-major table layout: half c holds rows [c*CHTOT, (c+1)*CHTOT)
    v = col_pad
    r = v // NLOC
    j = v % NLOC
    c = j // CHROWS
    w = j % CHROWS
    vr = (c * CHTOT + r * CHROWS + w).astype(np.int32)

    x = np.asarray(inputs, np.float32)
    x_pad = np.zeros((NPAD, IN_DIM), np.float32)
    x_pad[:N] = x

    Ws = [np.asarray(W0, np.float32), np.asarray(W1, np.float32),
          np.asarray(W2, np.float32)]
    As = [_pack_a(al0, ar0, D1, HID), _pack_a(al1, ar1, D1, HID),
          _pack_a(al2, ar2, D2, NCLS)]
    Wext = [np.concatenate([Ws[l], Ws[l] @ As[l]], axis=1).astype(BF16)
            for l in range(3)]

    in_maps = []
    for cc in range(NCORES):
        lo = cc * NLOC
        xT = x_pad[lo:lo + NLOC].T                       # [256, NLOC] f32
        # [fi, node] -> [p, (t, ic, n)] with fi = ic*128 + p
        h0 = xT.reshape(2, P, TPC, P).transpose(1, 2, 0, 3)
        h0 = np.ascontiguousarray(h0.reshape(P, TPC * 2 * P)).astype(BF16)
        ic = vr[lo:lo + NLOC]                            # [NLOC, 16]
        ia = ic.reshape(TPC, P, DEG).transpose(1, 0, 2).reshape(P, TPC * DEG)
        m = {"hT0": h0, "idx": np.ascontiguousarray(ia)}
        for l in range(3):
            m[f"Wx{l}"] = Wext[l]
        in_maps.append(m)
    return in_maps


_NC_CACHE = {}


def kernel(**inputs):
    from concourse.bass_utils import run_bass_kernel_spmd

    if "nc" not in _NC_CACHE:
        _NC_CACHE["nc"] = build_program()
    nc = _NC_CACHE["nc"]

    in_maps = prep_inputs(**inputs)

    trace = bool(int(os.environ.get("BASS_GAT_TRACE", "0")))
    res = run_bass_kernel_spmd(nc, in_maps, list(range(NCORES)), trace=trace)
    _NC_CACHE["last_exec_ns"] = res.exec_time_ns

    out = np.concatenate([res.results[c]["out"] for c in range(NCORES)],
                         axis=0)
    return np.ascontiguousarray(out[:N])


# revision 13
# speedup vs baseline: 1.0283x; 1.0283x over previous
"""GAT (3-layer, 4-head) on 8 Trainium2 NeuronCores.

Sharding: nodes padded to 100352 = 8 * 98 * 128; core c owns the contiguous
dst-node range [c*12544, (c+1)*12544) and its incoming-edge CSR slice.

Per layer: dense phase computes [feat | el | er] rows in one PSUM matmul
(hT stationary, Wext = [W | W@a_l | W@a_r] moving, bf16), writes packed
bf16 rows to DRAM; one AllGather per layer shares the table; the
aggregation phase gathers each dst tile's 16 neighbor rows with 16
single-offset indirect DMAs (the HW SWDGE indirect path supports one
offset per partition per instruction — multi-offset APs only work in the
simulator), then does edge-softmax + weighted sum on DVE. Layer l's
aggregation is fused per-tile with layer l+1's dense phase; gathers are
software-prefetched 3 tiles ahead of consumption.
"""
import os
import sys

sys.path.insert(0, "/opt/trn_rl_repo")

import numpy as np
import ml_dtypes

BF16 = ml_dtypes.bfloat16

P = 128
NCORES = 8
N = 100000
DEG = 16
HEADS = 4
HID = 64
IN_DIM = 256
NCLS = 41
NEG = 0.2

TPC = 98                  # dst tiles per core
NLOC = TPC * P            # 12544
NPAD = NCORES * NLOC      # 100352
D1 = HEADS * HID          # 256
D2 = HEADS * NCLS         # 164
ROW1 = D1 + 16            # packed row: 256 feat + 4 el + pad (544B)
ROW2 = D2 + 12            # 164 feat + 4 el + pad (352B)

NCH = 7                   # AllGather chunks per layer
CHT = TPC // NCH          # 14 tiles per chunk
CHROWS = CHT * P          # 1792 rows per core per chunk
CHTOT = NCORES * CHROWS   # 14336 rows per chunk in the gathered table

PREF = 6                  # gather software-prefetch depth (tiles)

DL = [D1, D1, D2]
ROWL = [ROW1, ROW1, ROW2]


def _pack_a(al, ar, fdim, hdim):
    a = np.zeros((fdim, 8), np.float32)
    al = np.asarray(al, np.float32)
    ar = np.asarray(ar, np.float32)
    for h in range(HEADS):
        a[h * hdim:(h + 1) * hdim, h] = al[h]
        a[h * hdim:(h + 1) * hdim, 4 + h] = ar[h]
    return a


def build_program():
    import concourse.bass as bass
    import concourse.bacc as bacc
    import concourse.mybir as mybir
    import concourse.tile as tile
    from concourse.masks import make_identity

    f32 = mybir.dt.float32
    bf16 = mybir.dt.bfloat16
    i32 = mybir.dt.int32
    ADD = mybir.AluOpType.add
    MULT = mybir.AluOpType.mult
    MAX = mybir.AluOpType.max
    nc = bacc.Bacc("TRN2", target_bir_lowering=False, debug=False,
                   num_devices=NCORES)

    hT0 = nc.declare_dram_parameter("hT0", [P, TPC * 2 * P], bf16,
                                    isOutput=False)
    idx_in = nc.declare_dram_parameter("idx", [P, TPC * DEG], i32,
                                       isOutput=False)
    Wx = [nc.declare_dram_parameter(f"Wx{l}", [2 * P, DL[l] + 8], bf16,
                                    isOutput=False) for l in range(3)]
    out_ext = nc.declare_dram_parameter("out", [NLOC, NCLS], f32,
                                        isOutput=True)

    rg = [list(range(NCORES))]

    with tile.TileContext(nc) as tc:
        with (
            tc.tile_pool(name="const", bufs=1) as cp,
            tc.tile_pool(name="resid", bufs=1) as rp,
            tc.tile_pool(name="wk", bufs=3) as wk,
            tc.tile_pool(name="gat", bufs=10) as gp,
            tc.tile_pool(name="seq", bufs=2) as sq,
            tc.tile_pool(name="psp", bufs=2, space="PSUM") as psp,
            tc.tile_pool(name="dram", bufs=1, space="DRAM") as dram,
        ):
            identb = cp.tile([P, P], bf16)
            make_identity(nc, identb[:])

            # weights resident in SBUF: wsb[l][ic] : [128, DL+8] bf16
            wsb = []
            for l in range(3):
                wl = []
                for ic in range(2):
                    w = cp.tile([P, DL[l] + 8], bf16, name=f"w{l}_{ic}")
                    nc.sync.dma_start(out=w[:],
                                      in_=Wx[l][ic * P:(ic + 1) * P, :])
                    wl.append(w)
                wsb.append(wl)

            # resident buffers
            h_res = rp.tile([P, TPC * D1], bf16)            # 6.3 MB
            er_res = [rp.tile([P, TPC * 4], bf16, name=f"er{l}")
                      for l in range(3)]
            idxs = rp.tile([P, TPC * DEG], i32)
            nc.sync.dma_start(out=idxs[:], in_=idx_in[:])

            # DRAM: AllGather inputs + per-layer gathered tables
            ag_in = [dram.tile([NLOC, ROWL[l]], bf16, name=f"ag{l}")
                     for l in range(3)]
            table = [dram.tile([NCH * CHTOT, ROWL[l]], bf16,
                               addr_space="Shared", name=f"table{l}")
                     for l in range(3)]

            def dense_tile(l, t):
                """feat/el/er for dst tile t of layer l; writes packed row
                table chunk + er_res; fires the chunk AllGather."""
                DO = DL[l]
                ROW = ROWL[l]
                ht = wk.tile([P, 2 * P], bf16, tag="ht", name=f"ht{l}_{t}")
                if l == 0:
                    nc.sync.dma_start(out=ht[:],
                                      in_=hT0[:, t * 2 * P:(t + 1) * 2 * P])
                else:
                    for ic in range(2):
                        tp = psp.tile([P, P], bf16, tag="tp", bufs=2,
                                      name=f"tp{l}_{t}_{ic}")
                        nc.tensor.transpose(
                            tp[:],
                            h_res[:, t * D1 + ic * P: t * D1 + (ic + 1) * P],
                            identb[:])
                        nc.scalar.copy(ht[:, ic * P:(ic + 1) * P], tp[:])
                fp = psp.tile([P, 272], f32, tag="fp", bufs=2,
                              name=f"fp{l}_{t}")
                for ic in range(2):
                    nc.tensor.matmul(fp[:, :DO + 8],
                                     ht[:, ic * P:(ic + 1) * P],
                                     wsb[l][ic][:, :DO + 8],
                                     start=(ic == 0), stop=(ic == 1))
                packed = wk.tile([P, ROW1], bf16, tag="pk", name=f"pk{l}_{t}")
                nc.scalar.copy(packed[:, :DO + 4], fp[:, :DO + 4])
                nc.scalar.copy(er_res[l][:, t * 4:(t + 1) * 4],
                               fp[:, DO + 4:DO + 8])
                nc.sync.dma_start(out=ag_in[l][t * P:(t + 1) * P, :],
                                  in_=packed[:, :ROW])
                if t == TPC - 1:
                    nc.gpsimd.collective_compute(
                        "AllGather", mybir.AluOpType.bypass,
                        replica_groups=rg,
                        ins=[ag_in[l][:]],
                        outs=[table[l][:]])

            def emit_gather(l, t):
                ROW = ROWL[l]
                G = gp.tile([P, DEG * ROW1], bf16, tag="G", name=f"G{l}_{t}")
                for k in range(DEG):
                    nc.gpsimd.indirect_dma_start(
                        out=G[:, k * ROW:(k + 1) * ROW],
                        out_offset=None,
                        in_=table[l][:],
                        in_offset=bass.IndirectOffsetOnAxis(
                            ap=idxs[:, t * DEG + k:t * DEG + k + 1], axis=0),
                    )
                return G

            def agg_tile(l, t, G):
                """edge softmax + weighted aggregation for dst tile t."""
                DO = DL[l]
                ROW = ROWL[l]
                hd = DO // HEADS
                Gv = G[:, :DEG * ROW].rearrange("p (k r) -> p k r", k=DEG)
                # e[p, k, h] = el_src + er_dst  (k-major so last dim packed)
                e = wk.tile([P, 64], f32, tag="e", name=f"e{l}_{t}")
                ev = e[:].rearrange("p (k h) -> p k h", k=DEG)
                nc.vector.tensor_tensor(
                    out=ev, in0=Gv[:, :, DO:DO + 4],
                    in1=er_res[l][:, t * 4:(t + 1) * 4].unsqueeze(1)
                        .to_broadcast([P, DEG, 4]),
                    op=ADD)
                # leaky relu: e = max(NEG*e, e)
                nc.vector.scalar_tensor_tensor(
                    out=e[:], in0=e[:], scalar=NEG, in1=e[:],
                    op0=MULT, op1=MAX)
                # exp + per-head denominators (ACT accumulates the sum)
                ex = wk.tile([P, 64], bf16, tag="ex", name=f"ex{l}_{t}")
                den = wk.tile([P, 4], f32, tag="den", name=f"den{l}_{t}")
                ev2 = e[:].rearrange("p (k h) -> p k h", k=DEG)
                exv = ex[:].rearrange("p (k h) -> p k h", k=DEG)
                for h in range(HEADS):
                    nc.scalar.activation(
                        exv[:, :, h], ev2[:, :, h],
                        mybir.ActivationFunctionType.Exp,
                        accum_out=den[:, h:h + 1])
                rden = wk.tile([P, 4], f32, tag="rden", name=f"rd{l}_{t}")
                nc.vector.reciprocal(rden[:], den[:])
                if l == 2:
                    nc.vector.tensor_scalar_mul(rden[:], rden[:],
                                                1.0 / HEADS)
                alp = wk.tile([P, 64], bf16, tag="alp", name=f"al{l}_{t}")
                nc.vector.tensor_tensor(
                    out=alp[:].rearrange("p (k h) -> p k h", k=DEG),
                    in0=exv,
                    in1=rden[:].unsqueeze(1).to_broadcast([P, DEG, 4]),
                    op=MULT)
                # msg[p, k, h, d] = feat * alpha
                W0 = DEG * DO
                msg = sq.tile([P, DEG * D1], bf16, tag="msg",
                              name=f"ms{l}_{t}")
                nc.vector.tensor_tensor(
                    out=msg[:, :W0].rearrange("p (k h d) -> p k h d",
                                              k=DEG, h=HEADS),
                    in0=Gv[:, :, 0:DO].rearrange("p k (h d) -> p k h d",
                                                 h=HEADS),
                    in1=alp[:].rearrange("p (k h) -> p k h", k=DEG)
                        .to_broadcast([P, DEG, 4, hd]),
                    op=MULT)
                # tree reduce over k: DVE does the two wide levels (as
                # scalar_tensor_tensor for the 4x perf mode), GpSimd the tail
                w1 = W0 // 2
                s0 = sq.tile([P, 2048], bf16, tag="s0", name=f"s0{l}_{t}")
                nc.vector.scalar_tensor_tensor(
                    out=s0[:, :w1], in0=msg[:, 0:w1], scalar=1.0,
                    in1=msg[:, w1:W0], op0=MULT, op1=ADD)
                w2 = w1 // 2
                s1 = sq.tile([P, 1024], bf16, tag="s1", name=f"s1{l}_{t}")
                nc.vector.scalar_tensor_tensor(
                    out=s1[:, :w2], in0=s0[:, 0:w2], scalar=1.0,
                    in1=s0[:, w2:w1], op0=MULT, op1=ADD)
                w3 = w2 // 2
                s2 = sq.tile([P, 512], bf16, tag="s2", name=f"s2{l}_{t}")
                nc.vector.scalar_tensor_tensor(
                    out=s2[:, :w3], in0=s1[:, 0:w3], scalar=1.0,
                    in1=s1[:, w3:w2], op0=MULT, op1=ADD)
                w4 = w3 // 2
                if l < 2:
                    nc.vector.scalar_tensor_tensor(
                        out=h_res[:, t * D1:(t + 1) * D1],
                        in0=s2[:, 0:w4], scalar=1.0,
                        in1=s2[:, w4:w3], op0=MULT, op1=ADD)
                else:
                    cur = wk.tile([P, D2], bf16, tag="cur", name=f"cu{t}")
                    nc.vector.scalar_tensor_tensor(
                        out=cur[:], in0=s2[:, 0:w4], scalar=1.0,
                        in1=s2[:, w4:w3], op0=MULT, op1=ADD)
                    lg = wk.tile([P, NCLS], f32, tag="lg", name=f"lg{t}")
                    nc.vector.tensor_reduce(
                        out=lg[:],
                        in_=cur[:].rearrange("p (h c) -> p c h", h=HEADS),
                        axis=mybir.AxisListType.X, op=ADD)
                    nc.sync.dma_start(out=out_ext[t * P:(t + 1) * P, :],
                                      in_=lg[:])

            # ---- stage 0: dense layer 0 + chunked AllGather of table0 ----
            for t in range(TPC):
                dense_tile(0, t)

            # ---- stages 1..3: agg(l) fused with dense(l+1) ----
            for l in range(3):
                pend = {}
                for tt in range(PREF):
                    pend[tt] = emit_gather(l, tt)
                for t in range(TPC):
                    if t + PREF < TPC:
                        pend[t + PREF] = emit_gather(l, t + PREF)
                    G = pend.pop(t)
                    agg_tile(l, t, G)
                    if l < 2:
                        dense_tile(l + 1, t)

    nc.compile()
    return nc


def prep_inputs(row_ptr, col_ind, inputs, W0, al0, ar0, W1, al1, ar1,
                W2, al2, ar2):
    col = np.asarray(col_ind, np.int32).reshape(N, DEG)
    col_pad = np.zeros((NPAD, DEG), np.int32)
    col_pad[:N] = col
    # single AllGather per layer: table rows are rank-major = node id
    vr = col_pad.astype(np.int32)

    x = np.asarray(inputs, np.float32)
    x_pad = np.zeros((NPAD, IN_DIM), np.float32)
    x_pad[:N] = x

    Ws = [np.asarray(W0, np.float32), np.asarray(W1, np.float32),
          np.asarray(W2, np.float32)]
    As = [_pack_a(al0, ar0, D1, HID), _pack_a(al1, ar1, D1, HID),
          _pack_a(al2, ar2, D2, NCLS)]
    Wext = [np.concatenate([Ws[l], Ws[l] @ As[l]], axis=1).astype(BF16)
            for l in range(3)]

    in_maps = []
    for cc in range(NCORES):
        lo = cc * NLOC
        xT = x_pad[lo:lo + NLOC].T                       # [256, NLOC] f32
        # [fi, node] -> [p, (t, ic, n)] with fi = ic*128 + p
        h0 = xT.reshape(2, P, TPC, P).transpose(1, 2, 0, 3)
        h0 = np.ascontiguousarray(h0.reshape(P, TPC * 2 * P)).astype(BF16)
        ic = vr[lo:lo + NLOC]                            # [NLOC, 16]
        ia = ic.reshape(TPC, P, DEG).transpose(1, 0, 2).reshape(P, TPC * DEG)
        m = {"hT0": h0, "idx": np.ascontiguousarray(ia)}
        for l in range(3):
            m[f"Wx{l}"] = Wext[l]
        in_maps.append(m)
    return in_maps


_NC_CACHE = {}


def kernel(**inputs):
    from concourse.bass_utils import run_bass_kernel_spmd

    if "nc" not in _NC_CACHE:
        _NC_CACHE["nc"] = build_program()
    nc = _NC_CACHE["nc"]

    in_maps = prep_inputs(**inputs)

    trace = bool(int(os.environ.get("BASS_GAT_TRACE", "0")))
    res = run_bass_kernel_spmd(nc, in_maps, list(range(NCORES)), trace=trace)
    _NC_CACHE["last_exec_ns"] = res.exec_time_ns

    out = np.concatenate([res.results[c]["out"] for c in range(NCORES)],
                         axis=0)
    return np.ascontiguousarray(out[:N])
